# revision 2
# baseline (speedup 1.0000x reference)
"""Trainium2 Bass kernel for nn_EnhancedBilinearInteraction.

Computes out[b, m] = sum_l tanh(bn(x)[b,l,m]) * tanh(bn(y)[b,l,m]) where bn is
training-mode batchnorm over (B, L) per feature m (biased variance).

Strategy (8 NeuronCores, data-parallel over B, B_loc = 8 per core):
  - Host supplies each core's shard twice: natural (l-major) layout for the
    stats pass, and an m-major transposed copy for the normalize/product pass
    (feature index on the SBUF partition axis), plus gamma/beta as [128, 2].
  - Pass 1 (stats): stream natural [128, 2048] tiles; ScalarE squares them;
    TensorE ones-matmuls accumulate per-feature sum / sumsq into PSUM
    (partition-axis contraction). Pure f32.
  - 4 KB AllReduce of (sum_x, sumsq_x, sum_y, sumsq_y) across the 8 cores.
  - Scale/bias: s = gamma * rsqrt(var + eps) (Sqrt + exact reciprocal + 2
    Newton refinements), b = beta - mean * s, laid out per-partition [128, 2].
  - Pass 2: stream m-major [128, 4096] tiles; one ScalarE op does
    tanh(s*x + b) in place (per-partition scale/bias); one VectorE
    scalar_tensor_tensor computes xb*yb with accum_out giving the partial
    L-sums directly. Final tiny PE transpose writes out (8, 256) per core.
"""
import numpy as np
from contextlib import ExitStack

import concourse.bass as bass
import concourse.bacc as bacc
import concourse.tile as tile
import concourse.mybir as mybir
from concourse.bass_utils import run_bass_kernel_spmd

F32 = mybir.dt.float32
AF = mybir.ActivationFunctionType
ALU = mybir.AluOpType

N_CORES = 8
B, L, M = 64, 8192, 256
B_LOC = B // N_CORES            # 8
N_TOTAL = float(B * L)          # 524288 elements per feature
EPS = 1e-5

LF1 = 2048                      # pass-1 tile free dim (1 MiB tiles)
NT1 = (B_LOC * L * M) // (128 * LF1)   # 64 tiles per tensor per core
SL1 = LF1 // 512                # 4 matmul slices per tile (fp32 moving max 512)
LF2 = 4096                      # pass-2 tile free dim (2 MiB tiles)
NLT = L // LF2                  # 2 l-tiles per (b, mc)

_NC_CACHE = {}


def _build_nc():
    if "nc" in _NC_CACHE:
        return _NC_CACHE["nc"]
    nc = bacc.Bacc("TRN2", target_bir_lowering=False, debug=False,
                   num_devices=N_CORES)

    x_nat = nc.dram_tensor("x_nat", [NT1, 128, LF1], F32, kind="ExternalInput")
    y_nat = nc.dram_tensor("y_nat", [NT1, 128, LF1], F32, kind="ExternalInput")
    x_t = nc.dram_tensor("x_t", [B_LOC, 2, 128, L], F32, kind="ExternalInput")
    y_t = nc.dram_tensor("y_t", [B_LOC, 2, 128, L], F32, kind="ExternalInput")
    gamma2 = nc.dram_tensor("gamma2", [128, 2], F32, kind="ExternalInput")
    beta2 = nc.dram_tensor("beta2", [128, 2], F32, kind="ExternalInput")
    out_d = nc.dram_tensor("out", [B_LOC, M], F32, kind="ExternalOutput")

    ones_d = nc.inline_tensor(np.ones((128, 1), np.float32), name="ones_c")
    ident_d = nc.inline_tensor(np.eye(128, dtype=np.float32), name="ident_c")

    with tile.TileContext(nc) as tc:
        with ExitStack() as ctx:
            const = ctx.enter_context(tc.tile_pool(name="const", bufs=1))
            p1x = ctx.enter_context(tc.tile_pool(name="p1x", bufs=3))
            p1y = ctx.enter_context(tc.tile_pool(name="p1y", bufs=3))
            p1sq = ctx.enter_context(tc.tile_pool(name="p1sq", bufs=2))
            pstat = ctx.enter_context(tc.tile_pool(name="pstat", bufs=1, space="PSUM"))
            small = ctx.enter_context(tc.tile_pool(name="small", bufs=1))
            dram = ctx.enter_context(tc.tile_pool(name="dramp", bufs=1, space="DRAM"))
            p2x = ctx.enter_context(tc.tile_pool(name="p2x", bufs=3))
            p2y = ctx.enter_context(tc.tile_pool(name="p2y", bufs=3))
            p2pr = ctx.enter_context(tc.tile_pool(name="p2pr", bufs=1))
            pout = ctx.enter_context(tc.tile_pool(name="pout", bufs=1, space="PSUM"))

            ones_sb = const.tile([128, 1], F32)
            nc.gpsimd.dma_start(ones_sb[:], ones_d.ap())
            ident_sb = const.tile([128, 128], F32)
            nc.gpsimd.dma_start(ident_sb[:], ident_d.ap())
            gamma_sb = const.tile([128, 2], F32)
            nc.gpsimd.dma_start(gamma_sb[:], gamma2.ap())
            beta_sb = const.tile([128, 2], F32)
            nc.gpsimd.dma_start(beta_sb[:], beta2.ap())

            # ---- pass 1: per-core per-feature sum and sumsq ----
            acc_sum_x = pstat.tile([1, 512], F32)
            acc_sq_x = pstat.tile([1, 512], F32)
            acc_sum_y = pstat.tile([1, 512], F32)
            acc_sq_y = pstat.tile([1, 512], F32)

            def stats_tile(t, src, pool, acc_sum, acc_sq, dma):
                tl = pool.tile([128, LF1], F32, name=f"t_{src.name}")
                dma(tl[:], src.ap()[t])
                first, last = t == 0, t == NT1 - 1
                for j in range(SL1):
                    nc.tensor.matmul(
                        acc_sum[:], ones_sb[:], tl[:, j * 512:(j + 1) * 512],
                        start=(first and j == 0), stop=(last and j == SL1 - 1),
                    )
                sq = p1sq.tile([128, LF1], F32, name="sqt")
                nc.scalar.activation(sq[:], tl[:], AF.Square)
                for j in range(SL1):
                    nc.tensor.matmul(
                        acc_sq[:], ones_sb[:], sq[:, j * 512:(j + 1) * 512],
                        start=(first and j == 0), stop=(last and j == SL1 - 1),
                    )

            for t in range(NT1):
                stats_tile(t, x_nat, p1x, acc_sum_x, acc_sq_x, nc.sync.dma_start)
                stats_tile(t, y_nat, p1y, acc_sum_y, acc_sq_y, nc.scalar.dma_start)

            # fold the two 256-wide halves of each [1, 512] accumulator and
            # write p-major into stats_sb: flat pos = p*8 + s*2 + c  (m = c*128+p)
            stats_sb = small.tile([1, 1024], F32)
            sview = stats_sb[:].rearrange("a (p s c) -> a s c p", p=128, s=4, c=2)
            for s, acc in enumerate([acc_sum_x, acc_sq_x, acc_sum_y, acc_sq_y]):
                tmp512 = small.tile([1, 512], F32, name=f"tmp512_{s}")
                nc.vector.tensor_copy(tmp512[:], acc[:])
                halves = tmp512[:].rearrange("a (r c p) -> r a c p", r=2, c=2, p=128)
                nc.vector.tensor_tensor(
                    sview[:, s], halves[0], halves[1], ALU.add)

            bounce_in = dram.tile([1, 1024], F32)
            bounce_out = dram.tile([1, 1024], F32)
            nc.gpsimd.dma_start(bounce_in[:], stats_sb[:])
            nc.gpsimd.collective_compute(
                "AllReduce", ALU.add,
                replica_groups=[list(range(N_CORES))],
                ins=[bounce_in.opt()], outs=[bounce_out.opt()],
            )
            statsT = small.tile([128, 8], F32)
            nc.gpsimd.dma_start(
                statsT[:], bounce_out[:].rearrange("a (p k) -> (a p) k", p=128, k=8))

            # ---- stats -> scale/bias, all [128, 2] per-partition ----
            def finalize(k_sum, k_sq):
                mean = small.tile([128, 2], F32, name=f"mean{k_sum}")
                nc.vector.tensor_scalar_mul(mean[:], statsT[:, k_sum:k_sum + 2], 1.0 / N_TOTAL)
                veps = small.tile([128, 2], F32, name=f"veps{k_sum}")
                nc.vector.tensor_scalar_mul(veps[:], statsT[:, k_sq:k_sq + 2], 1.0 / N_TOTAL)
                msq = small.tile([128, 2], F32, name=f"msq{k_sum}")
                nc.vector.tensor_tensor(msq[:], mean[:], mean[:], ALU.mult)
                nc.vector.tensor_tensor(veps[:], veps[:], msq[:], ALU.subtract)
                nc.vector.tensor_scalar_add(veps[:], veps[:], EPS)
                sq = small.tile([128, 2], F32, name=f"sqv{k_sum}")
                nc.scalar.activation(sq[:], veps[:], AF.Sqrt)
                r = small.tile([128, 2], F32, name=f"r{k_sum}")
                nc.vector.reciprocal(r[:], sq[:])
                tmp = small.tile([128, 2], F32, name=f"tmp{k_sum}")
                for _ in range(2):  # Newton rsqrt refinement (Sqrt table is loose)
                    nc.vector.tensor_tensor(tmp[:], r[:], r[:], ALU.mult)
                    nc.vector.tensor_tensor(tmp[:], tmp[:], veps[:], ALU.mult)
                    nc.vector.tensor_scalar(tmp[:], tmp[:], -0.5, 1.5, ALU.mult, ALU.add)
                    nc.vector.tensor_tensor(r[:], r[:], tmp[:], ALU.mult)
                s_t = small.tile([128, 2], F32, name=f"s{k_sum}")
                nc.vector.tensor_tensor(s_t[:], gamma_sb[:], r[:], ALU.mult)
                b_t = small.tile([128, 2], F32, name=f"b{k_sum}")
                nc.vector.tensor_tensor(b_t[:], mean[:], s_t[:], ALU.mult)
                nc.vector.tensor_tensor(b_t[:], beta_sb[:], b_t[:], ALU.subtract)
                return s_t, b_t

            s_x, b_x = finalize(0, 2)
            s_y, b_y = finalize(4, 6)

            # ---- pass 2: tanh-normalize, product, L-reduction ----
            acc = small.tile([128, B_LOC * 2 * NLT], F32)
            for b in range(B_LOC):
                for mc in range(2):
                    for lt in range(NLT):
                        xt2 = p2x.tile([128, LF2], F32, name="xt2")
                        nc.sync.dma_start(
                            xt2[:], x_t.ap()[b, mc, :, lt * LF2:(lt + 1) * LF2])
                        yt2 = p2y.tile([128, LF2], F32, name="yt2")
                        nc.scalar.dma_start(
                            yt2[:], y_t.ap()[b, mc, :, lt * LF2:(lt + 1) * LF2])
                        nc.scalar.activation(
                            xt2[:], xt2[:], AF.Tanh,
                            bias=b_x[:, mc:mc + 1], scale=s_x[:, mc:mc + 1])
                        nc.scalar.activation(
                            yt2[:], yt2[:], AF.Tanh,
                            bias=b_y[:, mc:mc + 1], scale=s_y[:, mc:mc + 1])
                        col = (b * 2 + mc) * NLT + lt
                        prod = p2pr.tile([128, LF2], F32, name="prod")
                        nc.vector.scalar_tensor_tensor(
                            prod[:], xt2[:], 1.0, yt2[:], ALU.mult, ALU.mult,
                            accum_out=acc[:, col:col + 1])

            red = small.tile([128, B_LOC * 2], F32)
            nc.vector.tensor_reduce(
                red[:], acc[:].rearrange("p (g lt) -> p g lt", lt=NLT),
                axis=mybir.AxisListType.X, op=ALU.add)
            outp = pout.tile([16, 128], F32)
            nc.tensor.transpose(outp[:], red[:], ident_sb[:])
            out_sb = small.tile([16, 128], F32)
            nc.vector.tensor_copy(out_sb[:], outp[:])
            nc.gpsimd.dma_start(
                out_d.ap().rearrange("b (mc p) -> (b mc) p", mc=2), out_sb[:])

    nc.compile()
    _NC_CACHE["nc"] = nc
    return nc


def kernel(x, y, gamma, beta):
    x = np.ascontiguousarray(np.asarray(x, dtype=np.float32))
    y = np.ascontiguousarray(np.asarray(y, dtype=np.float32))
    gamma2 = np.ascontiguousarray(
        np.asarray(gamma, dtype=np.float32).reshape(2, 128).T)
    beta2 = np.ascontiguousarray(
        np.asarray(beta, dtype=np.float32).reshape(2, 128).T)

    nc = _build_nc()
    in_maps = []
    for c in range(N_CORES):
        xs = x[c * B_LOC:(c + 1) * B_LOC]
        ys = y[c * B_LOC:(c + 1) * B_LOC]
        in_maps.append({
            "x_nat": xs.reshape(NT1, 128, LF1),
            "y_nat": ys.reshape(NT1, 128, LF1),
            "x_t": np.ascontiguousarray(xs.transpose(0, 2, 1)).reshape(B_LOC, 2, 128, L),
            "y_t": np.ascontiguousarray(ys.transpose(0, 2, 1)).reshape(B_LOC, 2, 128, L),
            "gamma2": gamma2,
            "beta2": beta2,
        })
    res = run_bass_kernel_spmd(nc, in_maps, core_ids=list(range(N_CORES)))
    return np.concatenate([res.results[c]["out"] for c in range(N_CORES)], axis=0)


# revision 7
# speedup vs baseline: 1.4072x; 1.4072x over previous
"""Trainium2 Bass kernel for nn_EnhancedBilinearInteraction.

Computes out[b, m] = sum_l tanh(bn(x)[b,l,m]) * tanh(bn(y)[b,l,m]) where bn is
training-mode batchnorm over (B, L) per feature m (biased variance).

Strategy (8 NeuronCores, data-parallel over B, B_loc = 8 per core):
  - Host supplies each core's shard twice: natural (l-major) layout for the
    stats pass, and an m-major transposed copy for the normalize/product pass
    (feature index on the SBUF partition axis), plus gamma/beta as [128, 2].
  - Pass 1 (stats): stream natural [128, 2048] tiles; ScalarE squares them;
    TensorE ones-matmuls accumulate per-feature sum / sumsq into PSUM
    (partition-axis contraction). Pure f32.
  - 4 KB AllReduce of (sum_x, sumsq_x, sum_y, sumsq_y) across the 8 cores.
  - Scale/bias: s = gamma * rsqrt(var + eps) (Sqrt + exact reciprocal + 2
    Newton refinements), b = beta - mean * s, laid out per-partition [128, 2].
  - Pass 2: stream m-major [128, 4096] tiles; one ScalarE op does
    tanh(s*x + b) in place (per-partition scale/bias); one VectorE
    scalar_tensor_tensor computes xb*yb with accum_out giving the partial
    L-sums directly. Final tiny PE transpose writes out (8, 256) per core.
"""
import numpy as np
from contextlib import ExitStack

import concourse.bass as bass
import concourse.bacc as bacc
import concourse.tile as tile
import concourse.mybir as mybir
from concourse.bass_utils import run_bass_kernel_spmd

F32 = mybir.dt.float32
BF16 = mybir.dt.bfloat16
AF = mybir.ActivationFunctionType
ALU = mybir.AluOpType

N_CORES = 8
B, L, M = 64, 8192, 256
B_LOC = B // N_CORES            # 8
N_TOTAL = float(B * L)          # 524288 elements per feature
EPS = 1e-5

LF1 = 2048                      # pass-1 tile free dim (1 MiB tiles)
NT1 = (B_LOC * L * M) // (128 * LF1)   # 64 tiles per tensor per core
SL1 = LF1 // 512                # 4 matmul slices per tile (fp32 moving max 512)
LF2 = 4096                      # pass-2 tile free dim (2 MiB tiles)
NLT = L // LF2                  # 2 l-tiles per (b, mc)

_NC_CACHE = {}


def _build_nc():
    if "nc" in _NC_CACHE:
        return _NC_CACHE["nc"]
    nc = bacc.Bacc("TRN2", target_bir_lowering=False, debug=False,
                   num_devices=N_CORES)

    x_nat = nc.dram_tensor("x_nat", [NT1, 128, LF1], F32, kind="ExternalInput")
    y_nat = nc.dram_tensor("y_nat", [NT1, 128, LF1], F32, kind="ExternalInput")
    x_t = nc.dram_tensor("x_t", [B_LOC, 2, 128, L], F32, kind="ExternalInput")
    y_t = nc.dram_tensor("y_t", [B_LOC, 2, 128, L], F32, kind="ExternalInput")
    gamma2 = nc.dram_tensor("gamma2", [128, 2], F32, kind="ExternalInput")
    beta2 = nc.dram_tensor("beta2", [128, 2], F32, kind="ExternalInput")
    out_d = nc.dram_tensor("out", [B_LOC, M], F32, kind="ExternalOutput")

    ones_d = nc.inline_tensor(np.ones((128, 1), np.float32), name="ones_c")
    ident_d = nc.inline_tensor(np.eye(128, dtype=np.float32), name="ident_c")

    with tile.TileContext(nc) as tc:
        with ExitStack() as ctx:
            const = ctx.enter_context(tc.tile_pool(name="const", bufs=1))
            p1x = ctx.enter_context(tc.tile_pool(name="p1x", bufs=3))
            p1y = ctx.enter_context(tc.tile_pool(name="p1y", bufs=3))
            p1sq = ctx.enter_context(tc.tile_pool(name="p1sq", bufs=2))
            pstat = ctx.enter_context(tc.tile_pool(name="pstat", bufs=1, space="PSUM"))
            small = ctx.enter_context(tc.tile_pool(name="small", bufs=1))
            dram = ctx.enter_context(tc.tile_pool(name="dramp", bufs=1, space="DRAM"))
            p2x = ctx.enter_context(tc.tile_pool(name="p2x", bufs=4))
            p2y = ctx.enter_context(tc.tile_pool(name="p2y", bufs=3))
            p2pr = ctx.enter_context(tc.tile_pool(name="p2pr", bufs=1))
            pout = ctx.enter_context(tc.tile_pool(name="pout", bufs=1, space="PSUM"))

            ones_sb = const.tile([128, 1], F32)
            nc.gpsimd.dma_start(ones_sb[:], ones_d.ap())
            ones_bf = const.tile([128, 1], BF16)
            nc.gpsimd.dma_start(ones_bf[:], ones_d.ap())  # SWDGE casts f32->bf16
            ident_sb = const.tile([128, 128], F32)
            nc.gpsimd.dma_start(ident_sb[:], ident_d.ap())
            gamma_sb = const.tile([128, 2], F32)
            nc.gpsimd.dma_start(gamma_sb[:], gamma2.ap())
            beta_sb = const.tile([128, 2], F32)
            nc.gpsimd.dma_start(beta_sb[:], beta2.ap())

            # ---- pass 1: per-core per-feature sum and sumsq ----
            # Tiles are cast f32->bf16 during the DMA (stat sums tolerate bf16
            # data: relative error ~1e-5 after averaging 512K elements).
            # x: TensorE ones-matmul contraction (bf16 weights avoid the fp32
            #    stationary penalty); y: VectorE strided partial-reduce over the
            #    8 rows per partition, then one small PE matmul per tile.
            acc_sum_x = pstat.tile([1, 512], F32)
            acc_sq_x = pstat.tile([1, 512], F32)
            acc_sum_y = pstat.tile([1, 256], F32)
            acc_sq_y = pstat.tile([1, 256], F32)
            R1 = LF1 // M               # rows folded per partition (8)

            def stats_tile_x(t):
                tl = p1x.tile([128, LF1], BF16, name="t_x")
                nc.gpsimd.dma_start(tl[:], x_nat.ap()[t])
                first, last = t == 0, t == NT1 - 1
                for j in range(SL1):
                    nc.tensor.matmul(
                        acc_sum_x[:], ones_bf[:], tl[:, j * 512:(j + 1) * 512],
                        start=(first and j == 0), stop=(last and j == SL1 - 1),
                    )
                sq = p1sq.tile([128, LF1], BF16, name="sqt")
                nc.scalar.activation(sq[:], tl[:], AF.Square)
                for j in range(SL1):
                    nc.tensor.matmul(
                        acc_sq_x[:], ones_bf[:], sq[:, j * 512:(j + 1) * 512],
                        start=(first and j == 0), stop=(last and j == SL1 - 1),
                    )

            def stats_tile_y(t):
                tl = p1y.tile([128, LF1], BF16, name="t_y")
                nc.gpsimd.dma_start(tl[:], y_nat.ap()[t])
                first, last = t == 0, t == NT1 - 1
                rview = tl[:].rearrange("p (r m) -> p m r", r=R1, m=M)
                part = p1sq.tile([128, M], F32, name="party")
                nc.vector.tensor_reduce(part[:], rview, axis=mybir.AxisListType.X, op=ALU.add)
                nc.tensor.matmul(acc_sum_y[:], ones_sb[:], part[:],
                                 start=first, stop=last)
                sq = p1sq.tile([128, LF1], BF16, name="sqty")
                nc.scalar.activation(sq[:], tl[:], AF.Square)
                sqv = sq[:].rearrange("p (r m) -> p m r", r=R1, m=M)
                part2 = p1sq.tile([128, M], F32, name="party2")
                nc.vector.tensor_reduce(part2[:], sqv, axis=mybir.AxisListType.X, op=ALU.add)
                nc.tensor.matmul(acc_sq_y[:], ones_sb[:], part2[:],
                                 start=first, stop=last)

            for t in range(NT1):
                stats_tile_x(t)
                stats_tile_y(t)

            # pack p-major into stats_sb: flat pos = p*8 + s*2 + c  (m = c*128+p)
            stats_sb = small.tile([1, 1024], F32)
            sview = stats_sb[:].rearrange("a (p s c) -> a s c p", p=128, s=4, c=2)
            for s, acc in [(0, acc_sum_x), (1, acc_sq_x)]:
                tmp512 = small.tile([1, 512], F32, name=f"tmp512_{s}")
                nc.vector.tensor_copy(tmp512[:], acc[:])
                halves = tmp512[:].rearrange("a (r c p) -> r a c p", r=2, c=2, p=128)
                nc.vector.tensor_tensor(
                    sview[:, s], halves[0], halves[1], ALU.add)
            for s, acc in [(2, acc_sum_y), (3, acc_sq_y)]:
                nc.vector.tensor_copy(
                    sview[:, s], acc[:].rearrange("a (c p) -> a c p", c=2, p=128))

            bounce_in = dram.tile([1, 1024], F32)
            bounce_out = dram.tile([1, 1024], F32)
            nc.gpsimd.dma_start(bounce_in[:], stats_sb[:])
            nc.gpsimd.collective_compute(
                "AllReduce", ALU.add,
                replica_groups=[list(range(N_CORES))],
                ins=[bounce_in.opt()], outs=[bounce_out.opt()],
            )
            statsT = small.tile([128, 8], F32)
            nc.gpsimd.dma_start(
                statsT[:], bounce_out[:].rearrange("a (p k) -> (a p) k", p=128, k=8))

            # ---- stats -> scale/bias, all [128, 2] per-partition ----
            def finalize(k_sum, k_sq):
                mean = small.tile([128, 2], F32, name=f"mean{k_sum}")
                nc.vector.tensor_scalar_mul(mean[:], statsT[:, k_sum:k_sum + 2], 1.0 / N_TOTAL)
                veps = small.tile([128, 2], F32, name=f"veps{k_sum}")
                nc.vector.tensor_scalar_mul(veps[:], statsT[:, k_sq:k_sq + 2], 1.0 / N_TOTAL)
                msq = small.tile([128, 2], F32, name=f"msq{k_sum}")
                nc.vector.tensor_tensor(msq[:], mean[:], mean[:], ALU.mult)
                nc.vector.tensor_tensor(veps[:], veps[:], msq[:], ALU.subtract)
                nc.vector.tensor_scalar_add(veps[:], veps[:], EPS)
                sq = small.tile([128, 2], F32, name=f"sqv{k_sum}")
                nc.scalar.activation(sq[:], veps[:], AF.Sqrt)
                r = small.tile([128, 2], F32, name=f"r{k_sum}")
                nc.vector.reciprocal(r[:], sq[:])
                tmp = small.tile([128, 2], F32, name=f"tmp{k_sum}")
                for _ in range(2):  # Newton rsqrt refinement (Sqrt table is loose)
                    nc.vector.tensor_tensor(tmp[:], r[:], r[:], ALU.mult)
                    nc.vector.tensor_tensor(tmp[:], tmp[:], veps[:], ALU.mult)
                    nc.vector.tensor_scalar(tmp[:], tmp[:], -0.5, 1.5, ALU.mult, ALU.add)
                    nc.vector.tensor_tensor(r[:], r[:], tmp[:], ALU.mult)
                s_t = small.tile([128, 2], F32, name=f"s{k_sum}")
                nc.vector.tensor_tensor(s_t[:], gamma_sb[:], r[:], ALU.mult)
                b_t = small.tile([128, 2], F32, name=f"b{k_sum}")
                nc.vector.tensor_tensor(b_t[:], mean[:], s_t[:], ALU.mult)
                nc.vector.tensor_tensor(b_t[:], beta_sb[:], b_t[:], ALU.subtract)
                return s_t, b_t

            s_x, b_x = finalize(0, 2)
            s_y, b_y = finalize(4, 6)

            # ---- pass 2: tanh-normalize, product, L-reduction ----
            acc = small.tile([128, B_LOC * 2 * NLT], F32)
            for b in range(B_LOC):
                for mc in range(2):
                    for lt in range(NLT):
                        xt2 = p2x.tile([128, LF2], F32, name="xt2")
                        nc.sync.dma_start(
                            xt2[:], x_t.ap()[b, mc, :, lt * LF2:(lt + 1) * LF2])
                        yt2 = p2y.tile([128, LF2], F32, name="yt2")
                        nc.scalar.dma_start(
                            yt2[:], y_t.ap()[b, mc, :, lt * LF2:(lt + 1) * LF2])
                        nc.scalar.activation(
                            xt2[:], xt2[:], AF.Tanh,
                            bias=b_x[:, mc:mc + 1], scale=s_x[:, mc:mc + 1])
                        nc.scalar.activation(
                            yt2[:], yt2[:], AF.Tanh,
                            bias=b_y[:, mc:mc + 1], scale=s_y[:, mc:mc + 1])
                        col = (b * 2 + mc) * NLT + lt
                        prod = p2pr.tile([128, LF2], F32, name="prod")
                        nc.vector.scalar_tensor_tensor(
                            prod[:], xt2[:], 1.0, yt2[:], ALU.mult, ALU.mult,
                            accum_out=acc[:, col:col + 1])

            red = small.tile([128, B_LOC * 2], F32)
            nc.vector.tensor_reduce(
                red[:], acc[:].rearrange("p (g lt) -> p g lt", lt=NLT),
                axis=mybir.AxisListType.X, op=ALU.add)
            outp = pout.tile([16, 128], F32)
            nc.tensor.transpose(outp[:], red[:], ident_sb[:])
            out_sb = small.tile([16, 128], F32)
            nc.vector.tensor_copy(out_sb[:], outp[:])
            nc.gpsimd.dma_start(
                out_d.ap().rearrange("b (mc p) -> (b mc) p", mc=2), out_sb[:])

    nc.compile()
    _NC_CACHE["nc"] = nc
    return nc


def kernel(x, y, gamma, beta):
    x = np.ascontiguousarray(np.asarray(x, dtype=np.float32))
    y = np.ascontiguousarray(np.asarray(y, dtype=np.float32))
    gamma2 = np.ascontiguousarray(
        np.asarray(gamma, dtype=np.float32).reshape(2, 128).T)
    beta2 = np.ascontiguousarray(
        np.asarray(beta, dtype=np.float32).reshape(2, 128).T)

    nc = _build_nc()
    in_maps = []
    for c in range(N_CORES):
        xs = x[c * B_LOC:(c + 1) * B_LOC]
        ys = y[c * B_LOC:(c + 1) * B_LOC]
        in_maps.append({
            "x_nat": xs.reshape(NT1, 128, LF1),
            "y_nat": ys.reshape(NT1, 128, LF1),
            "x_t": np.ascontiguousarray(xs.transpose(0, 2, 1)).reshape(B_LOC, 2, 128, L),
            "y_t": np.ascontiguousarray(ys.transpose(0, 2, 1)).reshape(B_LOC, 2, 128, L),
            "gamma2": gamma2,
            "beta2": beta2,
        })
    res = run_bass_kernel_spmd(nc, in_maps, core_ids=list(range(N_CORES)))
    return np.concatenate([res.results[c]["out"] for c in range(N_CORES)], axis=0)


# revision 8
# speedup vs baseline: 1.5284x; 1.0861x over previous
"""Trainium2 Bass kernel for nn_EnhancedBilinearInteraction.

Computes out[b, m] = sum_l tanh(bn(x)[b,l,m]) * tanh(bn(y)[b,l,m]) where bn is
training-mode batchnorm over (B, L) per feature m (biased variance).

Strategy (8 NeuronCores, data-parallel over B, B_loc = 8 per core):
  - Host supplies each core's shard twice: natural (l-major) layout for the
    stats pass, and an m-major transposed copy for the normalize/product pass
    (feature index on the SBUF partition axis), plus gamma/beta as [128, 2].
  - Pass 1 (stats): stream natural [128, 2048] tiles; ScalarE squares them;
    TensorE ones-matmuls accumulate per-feature sum / sumsq into PSUM
    (partition-axis contraction). Pure f32.
  - 4 KB AllReduce of (sum_x, sumsq_x, sum_y, sumsq_y) across the 8 cores.
  - Scale/bias: s = gamma * rsqrt(var + eps) (Sqrt + exact reciprocal + 2
    Newton refinements), b = beta - mean * s, laid out per-partition [128, 2].
  - Pass 2: stream m-major [128, 4096] tiles; one ScalarE op does
    tanh(s*x + b) in place (per-partition scale/bias); one VectorE
    scalar_tensor_tensor computes xb*yb with accum_out giving the partial
    L-sums directly. Final tiny PE transpose writes out (8, 256) per core.
"""
import numpy as np
from contextlib import ExitStack

import concourse.bass as bass
import concourse.bacc as bacc
import concourse.tile as tile
import concourse.mybir as mybir
from concourse.bass_utils import run_bass_kernel_spmd

F32 = mybir.dt.float32
BF16 = mybir.dt.bfloat16
AF = mybir.ActivationFunctionType
ALU = mybir.AluOpType

N_CORES = 8
B, L, M = 64, 8192, 256
B_LOC = B // N_CORES            # 8
N_TOTAL = float(B * L)          # 524288 elements per feature
EPS = 1e-5

LF1 = 2048                      # pass-1 tile free dim (1 MiB tiles)
NT1 = (B_LOC * L * M) // (128 * LF1)   # 64 tiles per tensor per core
SL1 = LF1 // 512                # 4 matmul slices per tile (fp32 moving max 512)
LF2 = 4096                      # pass-2 tile free dim (2 MiB tiles)
NLT = L // LF2                  # 2 l-tiles per (b, mc)

_NC_CACHE = {}


def _build_nc():
    if "nc" in _NC_CACHE:
        return _NC_CACHE["nc"]
    nc = bacc.Bacc("TRN2", target_bir_lowering=False, debug=False,
                   num_devices=N_CORES)

    x_nat = nc.dram_tensor("x_nat", [NT1, 128, LF1], F32, kind="ExternalInput")
    y_nat = nc.dram_tensor("y_nat", [NT1, 128, LF1], F32, kind="ExternalInput")
    x_t = nc.dram_tensor("x_t", [B_LOC, 2, 128, L], F32, kind="ExternalInput")
    y_t = nc.dram_tensor("y_t", [B_LOC, 2, 128, L], F32, kind="ExternalInput")
    gamma2 = nc.dram_tensor("gamma2", [128, 2], F32, kind="ExternalInput")
    beta2 = nc.dram_tensor("beta2", [128, 2], F32, kind="ExternalInput")
    out_d = nc.dram_tensor("out", [B_LOC, M], F32, kind="ExternalOutput")

    ones_d = nc.inline_tensor(np.ones((128, 1), np.float32), name="ones_c")
    ident_d = nc.inline_tensor(np.eye(128, dtype=np.float32), name="ident_c")

    with tile.TileContext(nc) as tc:
        with ExitStack() as ctx:
            const = ctx.enter_context(tc.tile_pool(name="const", bufs=1))
            p1x = ctx.enter_context(tc.tile_pool(name="p1x", bufs=3))
            p1y = ctx.enter_context(tc.tile_pool(name="p1y", bufs=3))
            p1sq = ctx.enter_context(tc.tile_pool(name="p1sq", bufs=2))
            pstat = ctx.enter_context(tc.tile_pool(name="pstat", bufs=1, space="PSUM"))
            small = ctx.enter_context(tc.tile_pool(name="small", bufs=1))
            dram = ctx.enter_context(tc.tile_pool(name="dramp", bufs=1, space="DRAM"))
            p2x = ctx.enter_context(tc.tile_pool(name="p2x", bufs=4))
            p2y = ctx.enter_context(tc.tile_pool(name="p2y", bufs=3))
            p2pr = ctx.enter_context(tc.tile_pool(name="p2pr", bufs=1))
            pout = ctx.enter_context(tc.tile_pool(name="pout", bufs=1, space="PSUM"))

            ones_sb = const.tile([128, 1], F32)
            nc.gpsimd.dma_start(ones_sb[:], ones_d.ap())
            ones_bf = const.tile([128, 1], BF16)
            nc.gpsimd.dma_start(ones_bf[:], ones_d.ap())  # SWDGE casts f32->bf16
            ident_sb = const.tile([128, 128], F32)
            nc.gpsimd.dma_start(ident_sb[:], ident_d.ap())
            gamma_sb = const.tile([128, 2], F32)
            nc.gpsimd.dma_start(gamma_sb[:], gamma2.ap())
            beta_sb = const.tile([128, 2], F32)
            nc.gpsimd.dma_start(beta_sb[:], beta2.ap())

            # ---- pass 1: per-core per-feature sum and sumsq ----
            # Tiles are cast f32->bf16 during the DMA (stat sums tolerate bf16
            # data: relative error ~1e-5 after averaging 512K elements).
            # x: TensorE ones-matmul contraction (bf16 weights avoid the fp32
            #    stationary penalty); y: VectorE strided partial-reduce over the
            #    8 rows per partition, then one small PE matmul per tile.
            acc_sum_x = pstat.tile([1, 512], F32)
            acc_sq_x = pstat.tile([1, 512], F32)
            acc_sum_y = pstat.tile([1, 512], F32)
            acc_sq_y = pstat.tile([1, 512], F32)
            R1 = LF1 // M               # rows per partition (8)

            def fold_and_mm(tl_ap, tag, acc, first, last):
                # [128, (8, 256)] -> two contiguous bf16 tree-fold adds ->
                # [128, (2, 256)] -> one bf16 ones-matmul into acc [1, 512]
                v8 = tl_ap.rearrange("p (r m) -> p r m", r=R1, m=M)
                f1 = p1sq.tile([128, 4 * M], BF16, name=f"f1{tag}")
                f1v = f1[:].rearrange("p (r m) -> p r m", r=4, m=M)
                nc.vector.tensor_tensor(f1v, v8[:, 0:4], v8[:, 4:8], ALU.add)
                f2 = p1sq.tile([128, 2 * M], BF16, name=f"f2{tag}")
                f2v = f2[:].rearrange("p (r m) -> p r m", r=2, m=M)
                nc.vector.tensor_tensor(f2v, f1v[:, 0:2], f1v[:, 2:4], ALU.add)
                nc.tensor.matmul(acc[:], ones_bf[:], f2[:], start=first, stop=last)

            def stats_tile(t, src, pool, tag, acc_sum, acc_sq):
                tl = pool.tile([128, LF1], BF16, name=f"t{tag}")
                nc.gpsimd.dma_start(tl[:], src.ap()[t])
                first, last = t == 0, t == NT1 - 1
                fold_and_mm(tl[:], tag, acc_sum, first, last)
                sq = p1sq.tile([128, LF1], BF16, name=f"sq{tag}")
                nc.scalar.activation(sq[:], tl[:], AF.Square)
                fold_and_mm(sq[:], "q" + tag, acc_sq, first, last)

            for t in range(NT1):
                stats_tile(t, x_nat, p1x, "x", acc_sum_x, acc_sq_x)
                stats_tile(t, y_nat, p1y, "y", acc_sum_y, acc_sq_y)

            # pack p-major into stats_sb: flat pos = p*8 + s*2 + c  (m = c*128+p)
            stats_sb = small.tile([1, 1024], F32)
            sview = stats_sb[:].rearrange("a (p s c) -> a s c p", p=128, s=4, c=2)
            for s, acc in enumerate([acc_sum_x, acc_sq_x, acc_sum_y, acc_sq_y]):
                tmp512 = small.tile([1, 512], F32, name=f"tmp512_{s}")
                nc.vector.tensor_copy(tmp512[:], acc[:])
                halves = tmp512[:].rearrange("a (r c p) -> r a c p", r=2, c=2, p=128)
                nc.vector.tensor_tensor(
                    sview[:, s], halves[0], halves[1], ALU.add)

            bounce_in = dram.tile([1, 1024], F32)
            bounce_out = dram.tile([1, 1024], F32)
            nc.gpsimd.dma_start(bounce_in[:], stats_sb[:])
            nc.gpsimd.collective_compute(
                "AllReduce", ALU.add,
                replica_groups=[list(range(N_CORES))],
                ins=[bounce_in.opt()], outs=[bounce_out.opt()],
            )
            statsT = small.tile([128, 8], F32)
            nc.gpsimd.dma_start(
                statsT[:], bounce_out[:].rearrange("a (p k) -> (a p) k", p=128, k=8))

            # ---- stats -> scale/bias, all [128, 2] per-partition ----
            def finalize(k_sum, k_sq):
                mean = small.tile([128, 2], F32, name=f"mean{k_sum}")
                nc.vector.tensor_scalar_mul(mean[:], statsT[:, k_sum:k_sum + 2], 1.0 / N_TOTAL)
                veps = small.tile([128, 2], F32, name=f"veps{k_sum}")
                nc.vector.tensor_scalar_mul(veps[:], statsT[:, k_sq:k_sq + 2], 1.0 / N_TOTAL)
                msq = small.tile([128, 2], F32, name=f"msq{k_sum}")
                nc.vector.tensor_tensor(msq[:], mean[:], mean[:], ALU.mult)
                nc.vector.tensor_tensor(veps[:], veps[:], msq[:], ALU.subtract)
                nc.vector.tensor_scalar_add(veps[:], veps[:], EPS)
                sq = small.tile([128, 2], F32, name=f"sqv{k_sum}")
                nc.scalar.activation(sq[:], veps[:], AF.Sqrt)
                r = small.tile([128, 2], F32, name=f"r{k_sum}")
                nc.vector.reciprocal(r[:], sq[:])
                tmp = small.tile([128, 2], F32, name=f"tmp{k_sum}")
                for _ in range(2):  # Newton rsqrt refinement (Sqrt table is loose)
                    nc.vector.tensor_tensor(tmp[:], r[:], r[:], ALU.mult)
                    nc.vector.tensor_tensor(tmp[:], tmp[:], veps[:], ALU.mult)
                    nc.vector.tensor_scalar(tmp[:], tmp[:], -0.5, 1.5, ALU.mult, ALU.add)
                    nc.vector.tensor_tensor(r[:], r[:], tmp[:], ALU.mult)
                s_t = small.tile([128, 2], F32, name=f"s{k_sum}")
                nc.vector.tensor_tensor(s_t[:], gamma_sb[:], r[:], ALU.mult)
                b_t = small.tile([128, 2], F32, name=f"b{k_sum}")
                nc.vector.tensor_tensor(b_t[:], mean[:], s_t[:], ALU.mult)
                nc.vector.tensor_tensor(b_t[:], beta_sb[:], b_t[:], ALU.subtract)
                return s_t, b_t

            s_x, b_x = finalize(0, 2)
            s_y, b_y = finalize(4, 6)

            # ---- pass 2: tanh-normalize, product, L-reduction ----
            acc = small.tile([128, B_LOC * 2 * NLT], F32)
            for b in range(B_LOC):
                for mc in range(2):
                    for lt in range(NLT):
                        xt2 = p2x.tile([128, LF2], F32, name="xt2")
                        nc.sync.dma_start(
                            xt2[:], x_t.ap()[b, mc, :, lt * LF2:(lt + 1) * LF2])
                        yt2 = p2y.tile([128, LF2], F32, name="yt2")
                        nc.scalar.dma_start(
                            yt2[:], y_t.ap()[b, mc, :, lt * LF2:(lt + 1) * LF2])
                        nc.scalar.activation(
                            xt2[:], xt2[:], AF.Tanh,
                            bias=b_x[:, mc:mc + 1], scale=s_x[:, mc:mc + 1])
                        nc.scalar.activation(
                            yt2[:], yt2[:], AF.Tanh,
                            bias=b_y[:, mc:mc + 1], scale=s_y[:, mc:mc + 1])
                        col = (b * 2 + mc) * NLT + lt
                        prod = p2pr.tile([128, LF2], F32, name="prod")
                        nc.vector.scalar_tensor_tensor(
                            prod[:], xt2[:], 1.0, yt2[:], ALU.mult, ALU.mult,
                            accum_out=acc[:, col:col + 1])

            red = small.tile([128, B_LOC * 2], F32)
            nc.vector.tensor_reduce(
                red[:], acc[:].rearrange("p (g lt) -> p g lt", lt=NLT),
                axis=mybir.AxisListType.X, op=ALU.add)
            outp = pout.tile([16, 128], F32)
            nc.tensor.transpose(outp[:], red[:], ident_sb[:])
            out_sb = small.tile([16, 128], F32)
            nc.vector.tensor_copy(out_sb[:], outp[:])
            nc.gpsimd.dma_start(
                out_d.ap().rearrange("b (mc p) -> (b mc) p", mc=2), out_sb[:])

    nc.compile()
    _NC_CACHE["nc"] = nc
    return nc


def kernel(x, y, gamma, beta):
    x = np.ascontiguousarray(np.asarray(x, dtype=np.float32))
    y = np.ascontiguousarray(np.asarray(y, dtype=np.float32))
    gamma2 = np.ascontiguousarray(
        np.asarray(gamma, dtype=np.float32).reshape(2, 128).T)
    beta2 = np.ascontiguousarray(
        np.asarray(beta, dtype=np.float32).reshape(2, 128).T)

    nc = _build_nc()
    in_maps = []
    for c in range(N_CORES):
        xs = x[c * B_LOC:(c + 1) * B_LOC]
        ys = y[c * B_LOC:(c + 1) * B_LOC]
        in_maps.append({
            "x_nat": xs.reshape(NT1, 128, LF1),
            "y_nat": ys.reshape(NT1, 128, LF1),
            "x_t": np.ascontiguousarray(xs.transpose(0, 2, 1)).reshape(B_LOC, 2, 128, L),
            "y_t": np.ascontiguousarray(ys.transpose(0, 2, 1)).reshape(B_LOC, 2, 128, L),
            "gamma2": gamma2,
            "beta2": beta2,
        })
    res = run_bass_kernel_spmd(nc, in_maps, core_ids=list(range(N_CORES)))
    return np.concatenate([res.results[c]["out"] for c in range(N_CORES)], axis=0)


# revision 12
# speedup vs baseline: 1.6959x; 1.1096x over previous
"""Trainium2 Bass kernel for nn_EnhancedBilinearInteraction.

Computes out[b, m] = sum_l tanh(bn(x)[b,l,m]) * tanh(bn(y)[b,l,m]) where bn is
training-mode batchnorm over (B, L) per feature m (biased variance).

Strategy (8 NeuronCores, data-parallel over B, B_loc = 8 per core):
  - Host supplies each core's shard twice: natural (l-major) layout for the
    stats pass, and an m-major transposed copy for the normalize/product pass
    (feature index on the SBUF partition axis), plus gamma/beta as [128, 2].
  - Pass 1 (stats): stream natural [128, 2048] tiles; ScalarE squares them;
    TensorE ones-matmuls accumulate per-feature sum / sumsq into PSUM
    (partition-axis contraction). Pure f32.
  - 4 KB AllReduce of (sum_x, sumsq_x, sum_y, sumsq_y) across the 8 cores.
  - Scale/bias: s = gamma * rsqrt(var + eps) (Sqrt + exact reciprocal + 2
    Newton refinements), b = beta - mean * s, laid out per-partition [128, 2].
  - Pass 2: stream m-major [128, 4096] tiles; one ScalarE op does
    tanh(s*x + b) in place (per-partition scale/bias); one VectorE
    scalar_tensor_tensor computes xb*yb with accum_out giving the partial
    L-sums directly. Final tiny PE transpose writes out (8, 256) per core.
"""
import numpy as np
from contextlib import ExitStack

import concourse.bass as bass
import concourse.bacc as bacc
import concourse.tile as tile
import concourse.mybir as mybir
from concourse.bass_utils import run_bass_kernel_spmd

F32 = mybir.dt.float32
BF16 = mybir.dt.bfloat16
AF = mybir.ActivationFunctionType
ALU = mybir.AluOpType

N_CORES = 8
B, L, M = 64, 8192, 256
B_LOC = B // N_CORES            # 8
N_TOTAL = float(B * L)          # 524288 elements per feature
EPS = 1e-5

LF1 = 2048                      # pass-1 tile free dim (1 MiB tiles)
NT1 = (B_LOC * L * M) // (128 * LF1)   # 64 tiles per tensor per core
SL1 = LF1 // 512                # 4 matmul slices per tile (fp32 moving max 512)
LF2 = 4096                      # pass-2 tile free dim (2 MiB tiles)
NLT = L // LF2                  # 2 l-tiles per (b, mc)

_NC_CACHE = {}


def _build_nc():
    if "nc" in _NC_CACHE:
        return _NC_CACHE["nc"]
    nc = bacc.Bacc("TRN2", target_bir_lowering=False, debug=False,
                   num_devices=N_CORES)

    x_nat = nc.dram_tensor("x_nat", [NT1, 128, LF1], BF16, kind="ExternalInput")
    y_nat = nc.dram_tensor("y_nat", [NT1, 128, LF1], BF16, kind="ExternalInput")
    x_t = nc.dram_tensor("x_t", [B_LOC, 2, 128, L], F32, kind="ExternalInput")
    y_t = nc.dram_tensor("y_t", [B_LOC, 2, 128, L], F32, kind="ExternalInput")
    gamma2 = nc.dram_tensor("gamma2", [128, 2], F32, kind="ExternalInput")
    beta2 = nc.dram_tensor("beta2", [128, 2], F32, kind="ExternalInput")
    out_d = nc.dram_tensor("out", [B_LOC, M], F32, kind="ExternalOutput")

    ones_d = nc.inline_tensor(np.ones((128, 1), np.float32), name="ones_c")
    ident_d = nc.inline_tensor(np.eye(128, dtype=np.float32), name="ident_c")

    with tile.TileContext(nc) as tc:
        with ExitStack() as ctx:
            const = ctx.enter_context(tc.tile_pool(name="const", bufs=1))
            p1x = ctx.enter_context(tc.tile_pool(name="p1x", bufs=3))
            p1y = ctx.enter_context(tc.tile_pool(name="p1y", bufs=3))
            p1sq = ctx.enter_context(tc.tile_pool(name="p1sq", bufs=2))
            pstat = ctx.enter_context(tc.tile_pool(name="pstat", bufs=1, space="PSUM"))
            small = ctx.enter_context(tc.tile_pool(name="small", bufs=1))
            dram = ctx.enter_context(tc.tile_pool(name="dramp", bufs=1, space="DRAM"))
            p2x = ctx.enter_context(tc.tile_pool(name="p2x", bufs=4))
            p2y = ctx.enter_context(tc.tile_pool(name="p2y", bufs=3))
            p2pr = ctx.enter_context(tc.tile_pool(name="p2pr", bufs=1))
            pout = ctx.enter_context(tc.tile_pool(name="pout", bufs=1, space="PSUM"))

            ones_sb = const.tile([128, 1], F32)
            nc.gpsimd.dma_start(ones_sb[:], ones_d.ap())
            ones_bf = const.tile([128, 1], BF16)
            nc.gpsimd.dma_start(ones_bf[:], ones_d.ap())  # SWDGE casts f32->bf16
            ident_sb = const.tile([128, 128], F32)
            nc.gpsimd.dma_start(ident_sb[:], ident_d.ap())
            gamma_sb = const.tile([128, 2], F32)
            nc.gpsimd.dma_start(gamma_sb[:], gamma2.ap())
            beta_sb = const.tile([128, 2], F32)
            nc.gpsimd.dma_start(beta_sb[:], beta2.ap())

            # ---- pass 1: per-core per-feature sum and sumsq ----
            # Tiles are cast f32->bf16 during the DMA (stat sums tolerate bf16
            # data: relative error ~1e-5 after averaging 512K elements).
            # x: TensorE ones-matmul contraction (bf16 weights avoid the fp32
            #    stationary penalty); y: VectorE strided partial-reduce over the
            #    8 rows per partition, then one small PE matmul per tile.
            acc_sum_x = pstat.tile([1, 512], F32)
            acc_sq_x = pstat.tile([1, 512], F32)
            acc_sum_y = pstat.tile([1, 512], F32)
            acc_sq_y = pstat.tile([1, 512], F32)
            R1 = LF1 // M               # rows per partition (8)

            def fold_chain(eng, tl_ap, acc, first, last):
                # in-place contiguous bf16 tree-fold (8,256)->(4,256)->(2,256)
                # on engine `eng`, then one bf16 ones-matmul into acc [1,512].
                v8 = tl_ap.rearrange("p (r m) -> p r m", r=R1, m=M)
                eng.tensor_tensor(v8[:, 0:4], v8[:, 0:4], v8[:, 4:8], ALU.add)
                eng.tensor_tensor(v8[:, 0:2], v8[:, 0:2], v8[:, 2:4], ALU.add)
                nc.tensor.matmul(acc[:], ones_bf[:], tl_ap[:, 0:2 * M],
                                 start=first, stop=last)

            def direct_mms(tl_ap, acc, first, last):
                for j in range(SL1):
                    nc.tensor.matmul(
                        acc[:], ones_bf[:], tl_ap[:, j * 512:(j + 1) * 512],
                        start=(first and j == 0), stop=(last and j == SL1 - 1))

            def stats_tile_x(t):
                tl = p1x.tile([128, LF1], BF16, name="tx")
                nc.sync.dma_start(tl[:], x_nat.ap()[t])
                first, last = t == 0, t == NT1 - 1
                direct_mms(tl[:], acc_sum_x, first, last)       # PE raw sums
                sq = p1sq.tile([128, LF1], BF16, name="sqx")
                nc.scalar.activation(sq[:], tl[:], AF.Square)   # ACT square
                fold_chain(nc.vector, sq[:], acc_sq_x, first, last)  # DVE folds

            def stats_tile_y(t):
                tl = p1y.tile([128, LF1], BF16, name="ty")
                nc.scalar.dma_start(tl[:], y_nat.ap()[t])
                first, last = t == 0, t == NT1 - 1
                sq = p1sq.tile([128, LF1], BF16, name="sqy")
                nc.vector.tensor_tensor(sq[:], tl[:], tl[:], ALU.mult)  # DVE square
                direct_mms(sq[:], acc_sq_y, first, last)        # PE sq sums
                fold_chain(nc.gpsimd, tl[:], acc_sum_y, first, last)  # GpSimd folds

            for t in range(NT1):
                stats_tile_x(t)
                stats_tile_y(t)

            # pack p-major into stats_sb: flat pos = p*8 + s*2 + c  (m = c*128+p)
            stats_sb = small.tile([1, 1024], F32)
            sview = stats_sb[:].rearrange("a (p s c) -> a s c p", p=128, s=4, c=2)
            for s, acc in enumerate([acc_sum_x, acc_sq_x, acc_sum_y, acc_sq_y]):
                tmp512 = small.tile([1, 512], F32, name=f"tmp512_{s}")
                nc.vector.tensor_copy(tmp512[:], acc[:])
                halves = tmp512[:].rearrange("a (r c p) -> r a c p", r=2, c=2, p=128)
                nc.vector.tensor_tensor(
                    sview[:, s], halves[0], halves[1], ALU.add)

            bounce_in = dram.tile([1, 1024], F32)
            bounce_out = dram.tile([1, 1024], F32)
            nc.gpsimd.dma_start(bounce_in[:], stats_sb[:])
            nc.gpsimd.collective_compute(
                "AllReduce", ALU.add,
                replica_groups=[list(range(N_CORES))],
                ins=[bounce_in.opt()], outs=[bounce_out.opt()],
            )
            statsT = small.tile([128, 8], F32)
            nc.gpsimd.dma_start(
                statsT[:], bounce_out[:].rearrange("a (p k) -> (a p) k", p=128, k=8))

            # ---- stats -> scale/bias, all [128, 2] per-partition ----
            def finalize(k_sum, k_sq):
                mean = small.tile([128, 2], F32, name=f"mean{k_sum}")
                nc.vector.tensor_scalar_mul(mean[:], statsT[:, k_sum:k_sum + 2], 1.0 / N_TOTAL)
                veps = small.tile([128, 2], F32, name=f"veps{k_sum}")
                nc.vector.tensor_scalar_mul(veps[:], statsT[:, k_sq:k_sq + 2], 1.0 / N_TOTAL)
                msq = small.tile([128, 2], F32, name=f"msq{k_sum}")
                nc.vector.tensor_tensor(msq[:], mean[:], mean[:], ALU.mult)
                nc.vector.tensor_tensor(veps[:], veps[:], msq[:], ALU.subtract)
                nc.vector.tensor_scalar_add(veps[:], veps[:], EPS)
                sq = small.tile([128, 2], F32, name=f"sqv{k_sum}")
                nc.scalar.activation(sq[:], veps[:], AF.Sqrt)
                r = small.tile([128, 2], F32, name=f"r{k_sum}")
                nc.vector.reciprocal(r[:], sq[:])
                tmp = small.tile([128, 2], F32, name=f"tmp{k_sum}")
                for _ in range(2):  # Newton rsqrt refinement (Sqrt table is loose)
                    nc.vector.tensor_tensor(tmp[:], r[:], r[:], ALU.mult)
                    nc.vector.tensor_tensor(tmp[:], tmp[:], veps[:], ALU.mult)
                    nc.vector.tensor_scalar(tmp[:], tmp[:], -0.5, 1.5, ALU.mult, ALU.add)
                    nc.vector.tensor_tensor(r[:], r[:], tmp[:], ALU.mult)
                s_t = small.tile([128, 2], F32, name=f"s{k_sum}")
                nc.vector.tensor_tensor(s_t[:], gamma_sb[:], r[:], ALU.mult)
                b_t = small.tile([128, 2], F32, name=f"b{k_sum}")
                nc.vector.tensor_tensor(b_t[:], mean[:], s_t[:], ALU.mult)
                nc.vector.tensor_tensor(b_t[:], beta_sb[:], b_t[:], ALU.subtract)
                return s_t, b_t

            s_x, b_x = finalize(0, 2)
            s_y, b_y = finalize(4, 6)

            # ---- pass 2: tanh-normalize, product, L-reduction ----
            acc = small.tile([128, B_LOC * 2 * NLT], F32)
            for b in range(B_LOC):
                for mc in range(2):
                    for lt in range(NLT):
                        xt2 = p2x.tile([128, LF2], F32, name="xt2")
                        nc.sync.dma_start(
                            xt2[:], x_t.ap()[b, mc, :, lt * LF2:(lt + 1) * LF2])
                        yt2 = p2y.tile([128, LF2], F32, name="yt2")
                        nc.scalar.dma_start(
                            yt2[:], y_t.ap()[b, mc, :, lt * LF2:(lt + 1) * LF2])
                        nc.scalar.activation(
                            xt2[:], xt2[:], AF.Tanh,
                            bias=b_x[:, mc:mc + 1], scale=s_x[:, mc:mc + 1])
                        nc.scalar.activation(
                            yt2[:], yt2[:], AF.Tanh,
                            bias=b_y[:, mc:mc + 1], scale=s_y[:, mc:mc + 1])
                        col = (b * 2 + mc) * NLT + lt
                        prod = p2pr.tile([128, LF2], BF16, name="prod")
                        nc.vector.scalar_tensor_tensor(
                            prod[:], xt2[:], 1.0, yt2[:], ALU.mult, ALU.mult,
                            accum_out=acc[:, col:col + 1])

            red = small.tile([128, B_LOC * 2], F32)
            nc.vector.tensor_reduce(
                red[:], acc[:].rearrange("p (g lt) -> p g lt", lt=NLT),
                axis=mybir.AxisListType.X, op=ALU.add)
            outp = pout.tile([16, 128], F32)
            nc.tensor.transpose(outp[:], red[:], ident_sb[:])
            out_sb = small.tile([16, 128], F32)
            nc.vector.tensor_copy(out_sb[:], outp[:])
            nc.gpsimd.dma_start(
                out_d.ap().rearrange("b (mc p) -> (b mc) p", mc=2), out_sb[:])

    nc.compile()
    _NC_CACHE["nc"] = nc
    return nc


def kernel(x, y, gamma, beta):
    x = np.ascontiguousarray(np.asarray(x, dtype=np.float32))
    y = np.ascontiguousarray(np.asarray(y, dtype=np.float32))
    gamma2 = np.ascontiguousarray(
        np.asarray(gamma, dtype=np.float32).reshape(2, 128).T)
    beta2 = np.ascontiguousarray(
        np.asarray(beta, dtype=np.float32).reshape(2, 128).T)

    import ml_dtypes
    bf16 = np.dtype(ml_dtypes.bfloat16)
    nc = _build_nc()
    in_maps = []
    for c in range(N_CORES):
        xs = x[c * B_LOC:(c + 1) * B_LOC]
        ys = y[c * B_LOC:(c + 1) * B_LOC]
        in_maps.append({
            "x_nat": xs.reshape(NT1, 128, LF1).astype(bf16),
            "y_nat": ys.reshape(NT1, 128, LF1).astype(bf16),
            "x_t": np.ascontiguousarray(xs.transpose(0, 2, 1)).reshape(B_LOC, 2, 128, L),
            "y_t": np.ascontiguousarray(ys.transpose(0, 2, 1)).reshape(B_LOC, 2, 128, L),
            "gamma2": gamma2,
            "beta2": beta2,
        })
    res = run_bass_kernel_spmd(nc, in_maps, core_ids=list(range(N_CORES)))
    return np.concatenate([res.results[c]["out"] for c in range(N_CORES)], axis=0)


# revision 17
# speedup vs baseline: 1.7487x; 1.0311x over previous
"""Trainium2 Bass kernel for nn_EnhancedBilinearInteraction.

Computes out[b, m] = sum_l tanh(bn(x)[b,l,m]) * tanh(bn(y)[b,l,m]) where bn is
training-mode batchnorm over (B, L) per feature m (biased variance).

Strategy (8 NeuronCores, data-parallel over B, B_loc = 8 per core):
  - Host supplies each core's shard twice: natural (l-major) layout for the
    stats pass, and an m-major transposed copy for the normalize/product pass
    (feature index on the SBUF partition axis), plus gamma/beta as [128, 2].
  - Pass 1 (stats): stream natural [128, 2048] tiles; ScalarE squares them;
    TensorE ones-matmuls accumulate per-feature sum / sumsq into PSUM
    (partition-axis contraction). Pure f32.
  - 4 KB AllReduce of (sum_x, sumsq_x, sum_y, sumsq_y) across the 8 cores.
  - Scale/bias: s = gamma * rsqrt(var + eps) (Sqrt + exact reciprocal + 2
    Newton refinements), b = beta - mean * s, laid out per-partition [128, 2].
  - Pass 2: stream m-major [128, 4096] tiles; one ScalarE op does
    tanh(s*x + b) in place (per-partition scale/bias); one VectorE
    scalar_tensor_tensor computes xb*yb with accum_out giving the partial
    L-sums directly. Final tiny PE transpose writes out (8, 256) per core.
"""
import numpy as np
from contextlib import ExitStack

import concourse.bass as bass
import concourse.bacc as bacc
import concourse.tile as tile
import concourse.mybir as mybir
from concourse.bass_utils import run_bass_kernel_spmd

F32 = mybir.dt.float32
BF16 = mybir.dt.bfloat16
AF = mybir.ActivationFunctionType
ALU = mybir.AluOpType

N_CORES = 8
B, L, M = 64, 8192, 256
B_LOC = B // N_CORES            # 8
N_TOTAL = float(B * L)          # 524288 elements per feature
EPS = 1e-5

LF1 = 2048                      # pass-1 tile free dim (1 MiB tiles)
NT1 = (B_LOC * L * M) // (128 * LF1)   # 64 tiles per tensor per core
SL1 = LF1 // 512                # 4 matmul slices per tile (fp32 moving max 512)
LF2 = 4096                      # pass-2 tile free dim (2 MiB tiles)
NLT = L // LF2                  # 2 l-tiles per (b, mc)

_NC_CACHE = {}


def _build_nc():
    if "nc" in _NC_CACHE:
        return _NC_CACHE["nc"]
    nc = bacc.Bacc("TRN2", target_bir_lowering=False, debug=False,
                   num_devices=N_CORES)

    x1m = nc.dram_tensor("x1m", [B_LOC, 2, 128, L], BF16, kind="ExternalInput")
    y_nat = nc.dram_tensor("y_nat", [NT1, 128, LF1], BF16, kind="ExternalInput")
    x_t = nc.dram_tensor("x_t", [B_LOC, 2, 128, L], F32, kind="ExternalInput")
    y_t = nc.dram_tensor("y_t", [B_LOC, 2, 128, L], F32, kind="ExternalInput")
    gamma2 = nc.dram_tensor("gamma2", [128, 2], F32, kind="ExternalInput")
    beta2 = nc.dram_tensor("beta2", [128, 2], F32, kind="ExternalInput")
    out_d = nc.dram_tensor("out", [B_LOC, M], F32, kind="ExternalOutput")

    ones_d = nc.inline_tensor(np.ones((128, 1), np.float32), name="ones_c")
    ident_d = nc.inline_tensor(np.eye(128, dtype=np.float32), name="ident_c")

    with tile.TileContext(nc) as tc:
        with ExitStack() as ctx:
            const = ctx.enter_context(tc.tile_pool(name="const", bufs=1))
            p1x = ctx.enter_context(tc.tile_pool(name="p1x", bufs=3))
            p1y = ctx.enter_context(tc.tile_pool(name="p1y", bufs=3))
            p1sq = ctx.enter_context(tc.tile_pool(name="p1sq", bufs=2))
            pstat = ctx.enter_context(tc.tile_pool(name="pstat", bufs=1, space="PSUM"))
            small = ctx.enter_context(tc.tile_pool(name="small", bufs=1))
            dram = ctx.enter_context(tc.tile_pool(name="dramp", bufs=1, space="DRAM"))
            p2x = ctx.enter_context(tc.tile_pool(name="p2x", bufs=4))
            p2y = ctx.enter_context(tc.tile_pool(name="p2y", bufs=3))
            p2pr = ctx.enter_context(tc.tile_pool(name="p2pr", bufs=1))
            pout = ctx.enter_context(tc.tile_pool(name="pout", bufs=1, space="PSUM"))

            ones_sb = const.tile([128, 1], F32)
            nc.gpsimd.dma_start(ones_sb[:], ones_d.ap())
            ones_bf = const.tile([128, 1], BF16)
            nc.gpsimd.dma_start(ones_bf[:], ones_d.ap())  # SWDGE casts f32->bf16
            ident_sb = const.tile([128, 128], F32)
            nc.gpsimd.dma_start(ident_sb[:], ident_d.ap())
            gamma_sb = const.tile([128, 2], F32)
            nc.gpsimd.dma_start(gamma_sb[:], gamma2.ap())
            beta_sb = const.tile([128, 2], F32)
            nc.gpsimd.dma_start(beta_sb[:], beta2.ap())

            # ---- pass 1: per-core per-feature sum and sumsq ----
            # x: m-major bf16 tiles; bn_stats fuses mean+M2 per partition
            #    (= per feature) in one VectorE stream; bn_aggr merges groups.
            # y: natural bf16 tiles; ScalarE square + TensorE ones-matmul
            #    contractions (per-feature sums land in PSUM [1, 512]).
            acc_sum_y = pstat.tile([1, 512], F32)
            acc_sq_y = pstat.tile([1, 512], F32)
            NXT = B_LOC * 2 * (L // LF1)   # 64 x-tiles; 32 per m-chunk
            GRP = LF1 // 512               # bn_stats calls per tile
            bnacc = [small.tile([128, (NXT // 2) * GRP * 6], F32, name=f"bnacc{c}")
                     for c in range(2)]

            def stats_tile_x(b, mc, lt, slot):
                tl = p1x.tile([128, LF1], BF16, name="tx")
                nc.sync.dma_start(tl[:], x1m.ap()[b, mc, :, lt * LF1:(lt + 1) * LF1])
                for k in range(GRP):
                    nc.vector.bn_stats(
                        bnacc[mc][:, (slot * GRP + k) * 6:(slot * GRP + k) * 6 + 6],
                        tl[:, k * 512:(k + 1) * 512])

            def direct_mms(tl_ap, acc, first, last):
                for j in range(SL1):
                    nc.tensor.matmul(
                        acc[:], ones_bf[:], tl_ap[:, j * 512:(j + 1) * 512],
                        start=(first and j == 0), stop=(last and j == SL1 - 1))

            def stats_tile_y(t):
                tl = p1y.tile([128, LF1], BF16, name="ty")
                nc.scalar.dma_start(tl[:], y_nat.ap()[t])
                first, last = t == 0, t == NT1 - 1
                direct_mms(tl[:], acc_sum_y, first, last)
                sq = p1sq.tile([128, LF1], BF16, name="sqy")
                nc.scalar.activation(sq[:], tl[:], AF.Square)
                direct_mms(sq[:], acc_sq_y, first, last)

            NLT1 = L // LF1
            for t in range(NT1):
                b, mc, lt = t // (2 * NLT1), (t // NLT1) % 2, t % NLT1
                stats_tile_x(b, mc, lt, (t // (2 * NLT1)) * NLT1 + t % NLT1)
                stats_tile_y(t)

            # local stats, all per-partition: statsL[:, s*2+mc]
            N_LOC = float(B_LOC * L)
            statsL = small.tile([128, 4], F32)
            for mc in range(2):
                mv = small.tile([128, 2], F32, name=f"mv{mc}")
                nc.vector.bn_aggr(mv[:], bnacc[mc][:])
                msq = small.tile([128, 1], F32, name=f"msq_x{mc}")
                nc.vector.tensor_tensor(msq[:], mv[:, 0:1], mv[:, 0:1], ALU.mult)
                nc.vector.tensor_tensor(msq[:], mv[:, 1:2], msq[:], ALU.add)
                nc.vector.tensor_scalar_mul(statsL[:, 2 + mc:3 + mc], msq[:], N_LOC)
                nc.vector.tensor_scalar_mul(statsL[:, mc:mc + 1], mv[:, 0:1], N_LOC)

            bounce_in = dram.tile([128, 8], F32)
            bounce_out = dram.tile([128, 8], F32)
            nc.gpsimd.dma_start(bounce_in[:, 0:4], statsL[:])
            # y accumulators: [1,512] = (r mod 2, m); fold halves -> [1,256]
            # (m = c*128 + p), packed p-major (pos = p*4 + s*2 + c) into a flat
            # row, then bounce via DRAM to scatter across partitions.
            yp = small.tile([1, 512], F32)
            ypv = yp[:].rearrange("a (p s c) -> a s c p", p=128, s=2, c=2)
            for s, acc in enumerate([acc_sum_y, acc_sq_y]):
                tmp512 = small.tile([1, 512], F32, name=f"tmp512_{s}")
                nc.vector.tensor_copy(tmp512[:], acc[:])
                halves = tmp512[:].rearrange("a (r c p) -> r a c p", r=2, c=2, p=128)
                nc.vector.tensor_tensor(ypv[:, s], halves[0], halves[1], ALU.add)
            yscratch = dram.tile([1, 512], F32)
            nc.gpsimd.dma_start(yscratch[:], yp[:])
            nc.gpsimd.dma_start(
                bounce_in[:, 4:8],
                yscratch[:].rearrange("a (p k) -> (a p) k", p=128, k=4))
            nc.gpsimd.collective_compute(
                "AllReduce", ALU.add,
                replica_groups=[list(range(N_CORES))],
                ins=[bounce_in.opt()], outs=[bounce_out.opt()],
            )
            statsT = small.tile([128, 8], F32)
            nc.gpsimd.dma_start(statsT[:], bounce_out[:])

            # ---- stats -> scale/bias, all [128, 2] per-partition ----
            def finalize(k_sum, k_sq):
                mean = small.tile([128, 2], F32, name=f"mean{k_sum}")
                nc.vector.tensor_scalar_mul(mean[:], statsT[:, k_sum:k_sum + 2], 1.0 / N_TOTAL)
                veps = small.tile([128, 2], F32, name=f"veps{k_sum}")
                nc.vector.tensor_scalar_mul(veps[:], statsT[:, k_sq:k_sq + 2], 1.0 / N_TOTAL)
                msq = small.tile([128, 2], F32, name=f"msq{k_sum}")
                nc.vector.tensor_tensor(msq[:], mean[:], mean[:], ALU.mult)
                nc.vector.tensor_tensor(veps[:], veps[:], msq[:], ALU.subtract)
                nc.vector.tensor_scalar_add(veps[:], veps[:], EPS)
                sq = small.tile([128, 2], F32, name=f"sqv{k_sum}")
                nc.scalar.activation(sq[:], veps[:], AF.Sqrt)
                r = small.tile([128, 2], F32, name=f"r{k_sum}")
                nc.vector.reciprocal(r[:], sq[:])
                tmp = small.tile([128, 2], F32, name=f"tmp{k_sum}")
                for _ in range(2):  # Newton rsqrt refinement (Sqrt table is loose)
                    nc.vector.tensor_tensor(tmp[:], r[:], r[:], ALU.mult)
                    nc.vector.tensor_tensor(tmp[:], tmp[:], veps[:], ALU.mult)
                    nc.vector.tensor_scalar(tmp[:], tmp[:], -0.5, 1.5, ALU.mult, ALU.add)
                    nc.vector.tensor_tensor(r[:], r[:], tmp[:], ALU.mult)
                s_t = small.tile([128, 2], F32, name=f"s{k_sum}")
                nc.vector.tensor_tensor(s_t[:], gamma_sb[:], r[:], ALU.mult)
                b_t = small.tile([128, 2], F32, name=f"b{k_sum}")
                nc.vector.tensor_tensor(b_t[:], mean[:], s_t[:], ALU.mult)
                nc.vector.tensor_tensor(b_t[:], beta_sb[:], b_t[:], ALU.subtract)
                return s_t, b_t

            s_x, b_x = finalize(0, 2)
            s_y, b_y = finalize(4, 6)

            # ---- pass 2: tanh-normalize, product, L-reduction ----
            acc = small.tile([128, B_LOC * 2 * NLT], F32)
            for b in range(B_LOC):
                for mc in range(2):
                    for lt in range(NLT):
                        xt2 = p2x.tile([128, LF2], F32, name="xt2")
                        nc.sync.dma_start(
                            xt2[:], x_t.ap()[b, mc, :, lt * LF2:(lt + 1) * LF2])
                        yt2 = p2y.tile([128, LF2], F32, name="yt2")
                        nc.scalar.dma_start(
                            yt2[:], y_t.ap()[b, mc, :, lt * LF2:(lt + 1) * LF2])
                        nc.scalar.activation(
                            xt2[:], xt2[:], AF.Tanh,
                            bias=b_x[:, mc:mc + 1], scale=s_x[:, mc:mc + 1])
                        nc.scalar.activation(
                            yt2[:], yt2[:], AF.Tanh,
                            bias=b_y[:, mc:mc + 1], scale=s_y[:, mc:mc + 1])
                        col = (b * 2 + mc) * NLT + lt
                        prod = p2pr.tile([128, LF2], BF16, name="prod")
                        nc.vector.scalar_tensor_tensor(
                            prod[:], xt2[:], 1.0, yt2[:], ALU.mult, ALU.mult,
                            accum_out=acc[:, col:col + 1])

            red = small.tile([128, B_LOC * 2], F32)
            nc.vector.tensor_reduce(
                red[:], acc[:].rearrange("p (g lt) -> p g lt", lt=NLT),
                axis=mybir.AxisListType.X, op=ALU.add)
            outp = pout.tile([16, 128], F32)
            nc.tensor.transpose(outp[:], red[:], ident_sb[:])
            out_sb = small.tile([16, 128], F32)
            nc.vector.tensor_copy(out_sb[:], outp[:])
            nc.gpsimd.dma_start(
                out_d.ap().rearrange("b (mc p) -> (b mc) p", mc=2), out_sb[:])

    nc.compile()
    _NC_CACHE["nc"] = nc
    return nc


def make_in_maps(inputs):
    import ml_dtypes
    bf16 = np.dtype(ml_dtypes.bfloat16)
    x = np.ascontiguousarray(np.asarray(inputs["x"], dtype=np.float32))
    y = np.ascontiguousarray(np.asarray(inputs["y"], dtype=np.float32))
    gamma2 = np.ascontiguousarray(
        np.asarray(inputs["gamma"], dtype=np.float32).reshape(2, 128).T)
    beta2 = np.ascontiguousarray(
        np.asarray(inputs["beta"], dtype=np.float32).reshape(2, 128).T)
    in_maps = []
    for c in range(N_CORES):
        xs = x[c * B_LOC:(c + 1) * B_LOC]
        ys = y[c * B_LOC:(c + 1) * B_LOC]
        x_t = np.ascontiguousarray(xs.transpose(0, 2, 1)).reshape(B_LOC, 2, 128, L)
        in_maps.append({
            "x1m": x_t.astype(bf16),
            "y_nat": ys.reshape(NT1, 128, LF1).astype(bf16),
            "x_t": x_t,
            "y_t": np.ascontiguousarray(ys.transpose(0, 2, 1)).reshape(B_LOC, 2, 128, L),
            "gamma2": gamma2,
            "beta2": beta2,
        })
    return in_maps


def kernel(x, y, gamma, beta):
    nc = _build_nc()
    in_maps = make_in_maps({"x": x, "y": y, "gamma": gamma, "beta": beta})
    res = run_bass_kernel_spmd(nc, in_maps, core_ids=list(range(N_CORES)))
    return np.concatenate([res.results[c]["out"] for c in range(N_CORES)], axis=0)
